# revision 14
# baseline (speedup 1.0000x reference)
"""Multi-head softmax attention (b=4, s=2048, d=1024, 16 heads) on 8 trn2 cores.

Sharding: 2D over (batch, head-half). Core c handles batch c//2, heads
[8*(c%2), 8*(c%2)+8). Each core computes its QKV projections, attention for
its 8 heads, and a partial output projection (row-parallel over its 512
attn-out columns). Host sums the two partials per batch.

Device layout (per core):
  - x^T [1024, 2048] streamed hidden-major; Q^T/K^T produced per head-pair as
    [128, 2048] tiles (2 heads x 64 dims on partitions); V produced
    token-major as 16 tiles [128 tok, 8 heads, 65] with a ones column fused
    (65th col) so the attn@V matmul also yields softmax denominators.
  - Scores computed transposed (S^T = K @ Q^T) so exp-ed scores feed attn@V
    as the moving operand with k on partitions. Softmax max-subtraction is
    skipped (scores are O(5), exp is safe in fp32).
  - All matmuls run in float32r (1 cycle/row at N>=512, ~1e-4 accuracy).
"""

import numpy as np

HIDDEN = 1024
SEQ = 2048
BATCH = 4
HEADS = 16
HG = 8  # heads per core
HD = 64  # head dim

_CACHE = {}
_TRACE = False  # test.py sets this for profiling runs
LAST_RESULT = None


def _build_nc():
    from collections import deque

    import concourse.mybir as mybir
    import concourse.tile as tile
    from concourse import bacc

    f32 = mybir.dt.float32
    f16 = mybir.dt.float16
    Exp = mybir.ActivationFunctionType.Exp

    nc = bacc.Bacc("TRN2", target_bir_lowering=False, debug=False)
    xT = nc.dram_tensor("xT", [HIDDEN, SEQ], f16, kind="ExternalInput").ap()
    wqk = nc.dram_tensor("wqk", [HIDDEN, 1024], f16, kind="ExternalInput").ap()
    wv = nc.dram_tensor("wv", [HIDDEN, 512], f16, kind="ExternalInput").ap()
    wout = nc.dram_tensor("wout", [512, HIDDEN], f16, kind="ExternalInput").ap()
    bqk = nc.dram_tensor("bqk", [128, 8], f32, kind="ExternalInput").ap()
    outp = nc.dram_tensor("outp", [SEQ, HIDDEN], f32, kind="ExternalOutput").ap()

    with tile.TileContext(nc) as tc:
        with (
            tc.tile_pool(name="persist", bufs=1) as pers,
            tc.tile_pool(name="pp", bufs=10) as pppool,
            tc.tile_pool(name="small", bufs=2) as small,
            tc.tile_pool(name="obuf", bufs=3) as obuf,
            tc.tile_pool(name="ps_sc", bufs=2, space="PSUM") as ps_sc,
            tc.tile_pool(name="ps_ac", bufs=1, space="PSUM") as ps_ac,
            tc.tile_pool(name="ps_aux", bufs=2, space="PSUM") as ps_aux,
        ):
            xt16 = pers.tile([128, 8, SEQ], f16, tag="xt16", name="xt16")
            wqk16 = pers.tile([128, 8, 1024], f16, tag="wqk16", name="wqk16")
            wv16 = pers.tile([128, 8, 512], f16, tag="wv16", name="wv16")
            qk = [pers.tile([128, SEQ], f16, tag=f"qk{i}", name=f"qk{i}") for i in range(8)]
            vt = [pers.tile([128, HG, 128], f16, tag=f"vt{i}", name=f"vt{i}") for i in range(16)]
            aot = [pers.tile([128, SEQ], f16, tag=f"aot{i}", name=f"aot{i}") for i in range(4)]
            wout_sb = [pers.tile([128, HIDDEN], f16, tag=f"wo{i}", name=f"wo{i}") for i in range(4)]
            bqk_sb = pers.tile([128, 8], f32, tag="bqk", name="bqk")
            ones8 = pers.tile([128, HG], f16, tag="ones8", name="ones8")

            # input DMAs; V-path first so V matmuls can start early
            for hc in range(8):
                nc.sync.dma_start(wv16[:, hc, :], wv[hc * 128 : (hc + 1) * 128, :])
            for tt in range(4):
                for hc in range(8):
                    nc.sync.dma_start(
                        xt16[:, hc, tt * 512 : (tt + 1) * 512],
                        xT[hc * 128 : (hc + 1) * 128, tt * 512 : (tt + 1) * 512],
                    )
            for hc in range(8):
                nc.sync.dma_start(wqk16[:, hc, :], wqk[hc * 128 : (hc + 1) * 128, :])
            nc.sync.dma_start(bqk_sb[:], bqk[:])
            for i in range(4):
                nc.sync.dma_start(wout_sb[i][:], wout[i * 128 : (i + 1) * 128, :])
            nc.vector.memset(ones8[:], 1.0)
            for t in range(16):
                nc.vector.memset(vt[t][:, :, HD + 1 : 128], 0.0)
                nc.vector.tensor_copy(vt[t][:, :, HD], ones8[:])

            def aux_psum():
                return ps_aux.tile([128, 512], f32, tag="aux", name="aux")

            # ---- emission-step builders (dripped between attention groups) ----
            def qk_ops(pair):
                ops = []
                for tt in range(4):
                    for ccx in (4 + pair, pair):
                        cell = {}

                        def mk_mm(hc, ccx=ccx, tt=tt, cell=cell):
                            def f():
                                if "ps" not in cell:
                                    cell["ps"] = aux_psum()
                                nc.tensor.matmul(
                                    cell["ps"][:],
                                    wqk16[:, hc, ccx * 128 : (ccx + 1) * 128],
                                    xt16[:, hc, tt * 512 : (tt + 1) * 512],
                                    start=(hc == 0),
                                    stop=(hc == 7),
                                )
                            return f

                        for hc in range(8):
                            ops.append(mk_mm(hc))

                        def mk_ev(ccx=ccx, tt=tt, cell=cell):
                            def f():
                                nc.vector.tensor_scalar_add(
                                    qk[ccx][:, tt * 512 : (tt + 1) * 512],
                                    cell["ps"][:],
                                    bqk_sb[:, ccx : ccx + 1],
                                )
                            return f

                        ops.append(mk_ev())
                return ops

            def outproj_ops(qt):
                ops = []
                for t4 in range(4):
                    tch = qt * 4 + t4
                    for nt_ in range(2):
                        cell = {}

                        def mk_mm(pair_, tch=tch, nt_=nt_, cell=cell):
                            def f():
                                if "ps" not in cell:
                                    cell["ps"] = aux_psum()
                                nc.tensor.matmul(
                                    cell["ps"][:],
                                    aot[pair_][:, tch * 128 : (tch + 1) * 128],
                                    wout_sb[pair_][:, nt_ * 512 : (nt_ + 1) * 512],
                                    start=(pair_ == 0),
                                    stop=(pair_ == 3),
                                )
                            return f

                        for pair_ in range(4):
                            ops.append(mk_mm(pair_))

                        def mk_out(tch=tch, nt_=nt_, cell=cell):
                            def f():
                                ot = obuf.tile([128, 512], f32, tag="ot", name="ot")
                                nc.scalar.copy(ot[:], cell["ps"][:])
                                nc.sync.dma_start(
                                    outp[
                                        tch * 128 : (tch + 1) * 128,
                                        nt_ * 512 : (nt_ + 1) * 512,
                                    ],
                                    ot[:],
                                )
                            return f

                        ops.append(mk_out())
                return ops

            pending = deque()

            def drip(n):
                for _ in range(n):
                    if not pending:
                        return
                    pending.popleft()()

            def pace(budget, g, total_groups=64):
                return (budget * (g + 1)) // total_groups - (budget * g) // total_groups

            # ---- V projection (token-major, ones column pre-set) ----
            for t in range(16):
                pv = aux_psum()
                for hc in range(8):
                    nc.tensor.matmul(
                        pv[:],
                        xt16[:, hc, t * 128 : (t + 1) * 128],
                        wv16[:, hc, :],
                        start=(hc == 0),
                        stop=(hc == 7),
                    )
                nc.vector.tensor_copy(
                    vt[t][:, :, 0:HD], pv[:].rearrange("p (h d) -> p h d", h=HG)
                )

            # ---- Q^T/K^T for pair 0 eagerly ----
            for op in qk_ops(0):
                op()

            # ---- attention, with next-pair QK and out-proj dripped in ----
            for pair in range(4):
                if pair < 3:
                    pending.extend(qk_ops(pair + 1))
                drip_budget = len(pending)
                gidx = 0
                qtile = qk[pair]
                ktile = qk[4 + pair]
                for qt in range(4):
                    accA = ps_ac.tile([128, 512], f32, tag="accA", name="accA")
                    accB = ps_ac.tile([128, 512], f32, tag="accB", name="accB")
                    for kc in range(16):
                        sc = ps_sc.tile([128, 1024], f32, tag="sc", name="sc")
                        nc.tensor.matmul(
                            sc[:, 0:512],
                            ktile[0:64, kc * 128 : (kc + 1) * 128],
                            qtile[0:64, qt * 512 : (qt + 1) * 512],
                            start=True,
                            stop=True,
                            tile_position=(0, 0),
                        )
                        nc.tensor.matmul(
                            sc[:, 512:1024],
                            ktile[64:128, kc * 128 : (kc + 1) * 128],
                            qtile[64:128, qt * 512 : (qt + 1) * 512],
                            start=True,
                            stop=True,
                            tile_position=(64, 0),
                        )
                        pp = pppool.tile([128, 1024], f16, tag="pp", name="pp")
                        nc.scalar.activation(pp[:], sc[:], Exp)
                        nc.tensor.matmul(
                            accA[:],
                            vt[kc][:, 2 * pair, :],
                            pp[:, 0:512],
                            start=(kc == 0),
                            stop=(kc == 15),
                        )
                        nc.tensor.matmul(
                            accB[:],
                            vt[kc][:, 2 * pair + 1, :],
                            pp[:, 512:1024],
                            start=(kc == 0),
                            stop=(kc == 15),
                        )
                        drip(pace(drip_budget, gidx) + (3 if pair == 3 else 0))
                        gidx += 1
                    # normalization: read PSUM out early so acc banks free fast
                    parts = []
                    for acc, row0 in ((accA, 0), (accB, 64)):
                        dn = small.tile([1, 512], f32, tag="dn", name="dn")
                        nc.vector.tensor_copy(dn[:], acc[64:65, :])
                        numer = small.tile([64, 512], f32, tag="numer", name="numer")
                        nc.vector.tensor_copy(numer[:], acc[0:64, :])
                        parts.append((dn, numer, row0))
                    for dn, numer, row0 in parts:
                        bc = small.tile([64, 512], f32, tag="bc", name="bc")
                        nc.gpsimd.partition_broadcast(bc[:], dn[:])
                        rc = small.tile([64, 512], f32, tag="rc", name="rc")
                        scr = small.tile([64, 512], f32, tag="scr", name="scr")
                        nc.vector.reciprocal_approx_accurate(rc[:], bc[:], scr[:])
                        nc.vector.tensor_mul(
                            aot[pair][row0 : row0 + 64, qt * 512 : (qt + 1) * 512],
                            numer[:],
                            rc[:],
                        )
                    if pair == 3:
                        pending.extend(outproj_ops(qt))
            while pending:
                pending.popleft()()
    nc.compile()
    return nc


def _get_nc():
    if "nc" not in _CACHE:
        _CACHE["nc"] = _build_nc()
    return _CACHE["nc"]


def kernel(x, W_qkv, b_qkv, W_out, b_out):
    global LAST_RESULT
    from concourse.bass_utils import run_bass_kernel_spmd

    x = np.asarray(x, dtype=np.float32)
    W_qkv = np.asarray(W_qkv, dtype=np.float32)
    b_qkv = np.asarray(b_qkv, dtype=np.float32)
    W_out = np.asarray(W_out, dtype=np.float32)
    b_out = np.asarray(b_out, dtype=np.float32)

    scale = 1.0 / np.sqrt(HD)
    # [hidden, 3, heads, hd]
    w4 = W_qkv.reshape(HIDDEN, 3, HEADS, HD)
    b4 = b_qkv.reshape(3, HEADS, HD)

    in_maps = []
    for c in range(8):
        b = c // 2
        g = c % 2
        hs = slice(g * HG, (g + 1) * HG)
        wq = (w4[:, 0, hs, :] * scale).reshape(HIDDEN, 512)
        wk = w4[:, 1, hs, :].reshape(HIDDEN, 512)
        wv_ = np.ascontiguousarray(w4[:, 2, hs, :].reshape(HIDDEN, 512)).astype(
            np.float16
        )
        wqk = np.ascontiguousarray(np.concatenate([wq, wk], axis=1)).astype(np.float16)
        bq = (b4[0, hs, :] * scale).reshape(512)
        bk = b4[1, hs, :].reshape(512)
        bqk = np.ascontiguousarray(
            np.concatenate([bq, bk]).reshape(8, 128).T
        ).astype(np.float32)
        wout_c = np.ascontiguousarray(W_out[g * 512 : (g + 1) * 512, :]).astype(
            np.float16
        )
        xT_b = np.ascontiguousarray(x[b].T).astype(np.float16)
        in_maps.append(
            {
                "xT": xT_b,
                "wqk": wqk,
                "wv": wv_,
                "wout": wout_c,
                "bqk": bqk,
            }
        )

    nc = _get_nc()
    res = run_bass_kernel_spmd(
        nc, in_maps, core_ids=list(range(8)), trace=_TRACE
    )
    LAST_RESULT = res

    # host reduction: sum the two head-group partials per batch; fold V-bias
    # and output bias (adding b_v to V shifts every attn output row by b_v,
    # which after the out-projection is the constant b_v @ W_out).
    bv_all = b_qkv[2 * HIDDEN : 3 * HIDDEN]
    const = (b_out + bv_all @ W_out).astype(np.float32)
    out = np.empty((BATCH, SEQ, HIDDEN), dtype=np.float32)
    for b in range(BATCH):
        out[b] = res.results[2 * b]["outp"] + res.results[2 * b + 1]["outp"] + const
    return out


# revision 15
# speedup vs baseline: 1.0014x; 1.0014x over previous
"""Multi-head softmax attention (b=4, s=2048, d=1024, 16 heads) on 8 trn2 cores.

Sharding: 2D over (batch, head-half). Core c handles batch c//2, heads
[8*(c%2), 8*(c%2)+8). Each core computes its QKV projections, attention for
its 8 heads, and a partial output projection (row-parallel over its 512
attn-out columns). Host sums the two partials per batch.

Device layout (per core):
  - x^T [1024, 2048] streamed hidden-major; Q^T/K^T produced per head-pair as
    [128, 2048] tiles (2 heads x 64 dims on partitions); V produced
    token-major as 16 tiles [128 tok, 8 heads, 65] with a ones column fused
    (65th col) so the attn@V matmul also yields softmax denominators.
  - Scores computed transposed (S^T = K @ Q^T) so exp-ed scores feed attn@V
    as the moving operand with k on partitions. Softmax max-subtraction is
    skipped (scores are O(5), exp is safe in fp32).
  - All matmuls run in float32r (1 cycle/row at N>=512, ~1e-4 accuracy).
"""

import numpy as np

HIDDEN = 1024
SEQ = 2048
BATCH = 4
HEADS = 16
HG = 8  # heads per core
HD = 64  # head dim

_CACHE = {}
_TRACE = False  # test.py sets this for profiling runs
LAST_RESULT = None


def _build_nc():
    from collections import deque

    import concourse.mybir as mybir
    import concourse.tile as tile
    from concourse import bacc

    f32 = mybir.dt.float32
    f16 = mybir.dt.float16
    Exp = mybir.ActivationFunctionType.Exp

    nc = bacc.Bacc("TRN2", target_bir_lowering=False, debug=False)
    xT = nc.dram_tensor("xT", [HIDDEN, SEQ], f16, kind="ExternalInput").ap()
    wqk = nc.dram_tensor("wqk", [HIDDEN, 1024], f16, kind="ExternalInput").ap()
    wv = nc.dram_tensor("wv", [HIDDEN, 512], f16, kind="ExternalInput").ap()
    wout = nc.dram_tensor("wout", [512, HIDDEN], f16, kind="ExternalInput").ap()
    bqk = nc.dram_tensor("bqk", [128, 8], f32, kind="ExternalInput").ap()
    outp = nc.dram_tensor("outp", [SEQ, HIDDEN], f32, kind="ExternalOutput").ap()

    with tile.TileContext(nc) as tc:
        with (
            tc.tile_pool(name="persist", bufs=1) as pers,
            tc.tile_pool(name="pp", bufs=10) as pppool,
            tc.tile_pool(name="small", bufs=2) as small,
            tc.tile_pool(name="obuf", bufs=3) as obuf,
            tc.tile_pool(name="ps_sc", bufs=2, space="PSUM") as ps_sc,
            tc.tile_pool(name="ps_ac", bufs=1, space="PSUM") as ps_ac,
            tc.tile_pool(name="ps_aux", bufs=2, space="PSUM") as ps_aux,
        ):
            xt16 = pers.tile([128, 8, SEQ], f16, tag="xt16", name="xt16")
            wqk16 = pers.tile([128, 8, 1024], f16, tag="wqk16", name="wqk16")
            wv16 = pers.tile([128, 8, 512], f16, tag="wv16", name="wv16")
            qk = [pers.tile([128, SEQ], f16, tag=f"qk{i}", name=f"qk{i}") for i in range(8)]
            vt = [pers.tile([128, HG, 128], f16, tag=f"vt{i}", name=f"vt{i}") for i in range(16)]
            aot = [pers.tile([128, SEQ], f16, tag=f"aot{i}", name=f"aot{i}") for i in range(4)]
            wout_sb = [pers.tile([128, HIDDEN], f16, tag=f"wo{i}", name=f"wo{i}") for i in range(4)]
            bqk_sb = pers.tile([128, 8], f32, tag="bqk", name="bqk")
            ones8 = pers.tile([128, HG], f16, tag="ones8", name="ones8")

            # input DMAs; V-path first so V matmuls can start early
            for hc in range(8):
                nc.sync.dma_start(wv16[:, hc, :], wv[hc * 128 : (hc + 1) * 128, :])
            for tt in range(4):
                for hc in range(8):
                    nc.sync.dma_start(
                        xt16[:, hc, tt * 512 : (tt + 1) * 512],
                        xT[hc * 128 : (hc + 1) * 128, tt * 512 : (tt + 1) * 512],
                    )
            for hc in range(8):
                nc.sync.dma_start(wqk16[:, hc, :], wqk[hc * 128 : (hc + 1) * 128, :])
            nc.sync.dma_start(bqk_sb[:], bqk[:])
            for i in range(4):
                nc.sync.dma_start(wout_sb[i][:], wout[i * 128 : (i + 1) * 128, :])
            nc.vector.memset(ones8[:], 1.0)
            for t in range(16):
                nc.vector.memset(vt[t][:, :, HD + 1 : 128], 0.0)
                nc.vector.tensor_copy(vt[t][:, :, HD], ones8[:])

            def aux_psum():
                return ps_aux.tile([128, 512], f32, tag="aux", name="aux")

            # ---- emission-step builders (dripped between attention groups) ----
            def qk_ops(pair):
                ops = []
                for tt in range(4):
                    for ccx in (4 + pair, pair):
                        cell = {}

                        def mk_mm(hc, ccx=ccx, tt=tt, cell=cell):
                            def f():
                                if "ps" not in cell:
                                    cell["ps"] = aux_psum()
                                nc.tensor.matmul(
                                    cell["ps"][:],
                                    wqk16[:, hc, ccx * 128 : (ccx + 1) * 128],
                                    xt16[:, hc, tt * 512 : (tt + 1) * 512],
                                    start=(hc == 0),
                                    stop=(hc == 7),
                                )
                            return f

                        for hc in range(8):
                            ops.append(mk_mm(hc))

                        def mk_ev(ccx=ccx, tt=tt, cell=cell):
                            def f():
                                nc.vector.tensor_scalar_add(
                                    qk[ccx][:, tt * 512 : (tt + 1) * 512],
                                    cell["ps"][:],
                                    bqk_sb[:, ccx : ccx + 1],
                                )
                            return f

                        ops.append(mk_ev())
                return ops

            def outproj_ops(qt):
                ops = []
                for t4 in range(4):
                    tch = qt * 4 + t4
                    for nt_ in range(2):
                        cell = {}

                        def mk_mm(pair_, tch=tch, nt_=nt_, cell=cell):
                            def f():
                                if "ps" not in cell:
                                    cell["ps"] = aux_psum()
                                nc.tensor.matmul(
                                    cell["ps"][:],
                                    aot[pair_][:, tch * 128 : (tch + 1) * 128],
                                    wout_sb[pair_][:, nt_ * 512 : (nt_ + 1) * 512],
                                    start=(pair_ == 0),
                                    stop=(pair_ == 3),
                                )
                            return f

                        for pair_ in range(4):
                            ops.append(mk_mm(pair_))

                        def mk_out(tch=tch, nt_=nt_, cell=cell):
                            def f():
                                ot = obuf.tile([128, 512], f32, tag="ot", name="ot")
                                nc.vector.tensor_copy(ot[:], cell["ps"][:])
                                nc.sync.dma_start(
                                    outp[
                                        tch * 128 : (tch + 1) * 128,
                                        nt_ * 512 : (nt_ + 1) * 512,
                                    ],
                                    ot[:],
                                )
                            return f

                        ops.append(mk_out())
                return ops

            pending = deque()

            def drip(n):
                for _ in range(n):
                    if not pending:
                        return
                    pending.popleft()()

            def pace(budget, g, total_groups=64):
                return (budget * (g + 1)) // total_groups - (budget * g) // total_groups

            # ---- V projection (token-major, ones column pre-set) ----
            for t in range(16):
                pv = aux_psum()
                for hc in range(8):
                    nc.tensor.matmul(
                        pv[:],
                        xt16[:, hc, t * 128 : (t + 1) * 128],
                        wv16[:, hc, :],
                        start=(hc == 0),
                        stop=(hc == 7),
                    )
                nc.vector.tensor_copy(
                    vt[t][:, :, 0:HD], pv[:].rearrange("p (h d) -> p h d", h=HG)
                )

            # ---- Q^T/K^T for pair 0 eagerly ----
            for op in qk_ops(0):
                op()

            # ---- attention, with next-pair QK and out-proj dripped in ----
            for pair in range(4):
                if pair < 3:
                    pending.extend(qk_ops(pair + 1))
                drip_budget = len(pending)
                gidx = 0
                qtile = qk[pair]
                ktile = qk[4 + pair]
                for qt in range(4):
                    accA = ps_ac.tile([128, 512], f32, tag="accA", name="accA")
                    accB = ps_ac.tile([128, 512], f32, tag="accB", name="accB")
                    for kc in range(16):
                        sc = ps_sc.tile([128, 1024], f32, tag="sc", name="sc")
                        nc.tensor.matmul(
                            sc[:, 0:512],
                            ktile[0:64, kc * 128 : (kc + 1) * 128],
                            qtile[0:64, qt * 512 : (qt + 1) * 512],
                            start=True,
                            stop=True,
                            tile_position=(0, 0),
                        )
                        nc.tensor.matmul(
                            sc[:, 512:1024],
                            ktile[64:128, kc * 128 : (kc + 1) * 128],
                            qtile[64:128, qt * 512 : (qt + 1) * 512],
                            start=True,
                            stop=True,
                            tile_position=(64, 0),
                        )
                        pp = pppool.tile([128, 1024], f16, tag="pp", name="pp")
                        nc.scalar.activation(pp[:], sc[:], Exp)
                        nc.tensor.matmul(
                            accA[:],
                            vt[kc][:, 2 * pair, :],
                            pp[:, 0:512],
                            start=(kc == 0),
                            stop=(kc == 15),
                        )
                        nc.tensor.matmul(
                            accB[:],
                            vt[kc][:, 2 * pair + 1, :],
                            pp[:, 512:1024],
                            start=(kc == 0),
                            stop=(kc == 15),
                        )
                        drip(pace(drip_budget, gidx) + (3 if pair == 3 else 0))
                        gidx += 1
                    # normalization: read PSUM out early so acc banks free fast
                    parts = []
                    for acc, row0 in ((accA, 0), (accB, 64)):
                        dn = small.tile([1, 512], f32, tag="dn", name="dn")
                        nc.vector.tensor_copy(dn[:], acc[64:65, :])
                        numer = small.tile([64, 512], f32, tag="numer", name="numer")
                        nc.vector.tensor_copy(numer[:], acc[0:64, :])
                        parts.append((dn, numer, row0))
                    for dn, numer, row0 in parts:
                        bc = small.tile([64, 512], f32, tag="bc", name="bc")
                        nc.gpsimd.partition_broadcast(bc[:], dn[:])
                        rc = small.tile([64, 512], f32, tag="rc", name="rc")
                        scr = small.tile([64, 512], f32, tag="scr", name="scr")
                        nc.vector.reciprocal_approx_accurate(rc[:], bc[:], scr[:])
                        nc.vector.tensor_mul(
                            aot[pair][row0 : row0 + 64, qt * 512 : (qt + 1) * 512],
                            numer[:],
                            rc[:],
                        )
                    if pair == 3:
                        pending.extend(outproj_ops(qt))
            while pending:
                pending.popleft()()
    nc.compile()
    return nc


def _get_nc():
    if "nc" not in _CACHE:
        _CACHE["nc"] = _build_nc()
    return _CACHE["nc"]


def kernel(x, W_qkv, b_qkv, W_out, b_out):
    global LAST_RESULT
    from concourse.bass_utils import run_bass_kernel_spmd

    x = np.asarray(x, dtype=np.float32)
    W_qkv = np.asarray(W_qkv, dtype=np.float32)
    b_qkv = np.asarray(b_qkv, dtype=np.float32)
    W_out = np.asarray(W_out, dtype=np.float32)
    b_out = np.asarray(b_out, dtype=np.float32)

    scale = 1.0 / np.sqrt(HD)
    # [hidden, 3, heads, hd]
    w4 = W_qkv.reshape(HIDDEN, 3, HEADS, HD)
    b4 = b_qkv.reshape(3, HEADS, HD)

    in_maps = []
    for c in range(8):
        b = c // 2
        g = c % 2
        hs = slice(g * HG, (g + 1) * HG)
        wq = (w4[:, 0, hs, :] * scale).reshape(HIDDEN, 512)
        wk = w4[:, 1, hs, :].reshape(HIDDEN, 512)
        wv_ = np.ascontiguousarray(w4[:, 2, hs, :].reshape(HIDDEN, 512)).astype(
            np.float16
        )
        wqk = np.ascontiguousarray(np.concatenate([wq, wk], axis=1)).astype(np.float16)
        bq = (b4[0, hs, :] * scale).reshape(512)
        bk = b4[1, hs, :].reshape(512)
        bqk = np.ascontiguousarray(
            np.concatenate([bq, bk]).reshape(8, 128).T
        ).astype(np.float32)
        wout_c = np.ascontiguousarray(W_out[g * 512 : (g + 1) * 512, :]).astype(
            np.float16
        )
        xT_b = np.ascontiguousarray(x[b].T).astype(np.float16)
        in_maps.append(
            {
                "xT": xT_b,
                "wqk": wqk,
                "wv": wv_,
                "wout": wout_c,
                "bqk": bqk,
            }
        )

    nc = _get_nc()
    res = run_bass_kernel_spmd(
        nc, in_maps, core_ids=list(range(8)), trace=_TRACE
    )
    LAST_RESULT = res

    # host reduction: sum the two head-group partials per batch; fold V-bias
    # and output bias (adding b_v to V shifts every attn output row by b_v,
    # which after the out-projection is the constant b_v @ W_out).
    bv_all = b_qkv[2 * HIDDEN : 3 * HIDDEN]
    const = (b_out + bv_all @ W_out).astype(np.float32)
    out = np.empty((BATCH, SEQ, HIDDEN), dtype=np.float32)
    for b in range(BATCH):
        out[b] = res.results[2 * b]["outp"] + res.results[2 * b + 1]["outp"] + const
    return out


# revision 16
# speedup vs baseline: 1.0045x; 1.0031x over previous
"""Multi-head softmax attention (b=4, s=2048, d=1024, 16 heads) on 8 trn2 cores.

Sharding: 2D over (batch, head-half). Core c handles batch c//2, heads
[8*(c%2), 8*(c%2)+8). Each core computes its QKV projections, attention for
its 8 heads, and a partial output projection (row-parallel over its 512
attn-out columns). Host sums the two partials per batch.

Device layout (per core):
  - x^T [1024, 2048] streamed hidden-major; Q^T/K^T produced per head-pair as
    [128, 2048] tiles (2 heads x 64 dims on partitions); V produced
    token-major as 16 tiles [128 tok, 8 heads, 65] with a ones column fused
    (65th col) so the attn@V matmul also yields softmax denominators.
  - Scores computed transposed (S^T = K @ Q^T, row-tiled so both heads of a
    pair run concurrently on the PE) so exp-ed scores feed attn@V as the
    moving operand with k on partitions. Softmax max-subtraction is skipped
    (scores are O(5), exp is safe well within fp16/fp32 range).
  - Data path is fp16 (weights/activations/P'), accumulation fp32 in PSUM,
    softmax normalization in fp32: measured ~6.5e-4 relative error.
  - Emission interleave: the next pair's Q^T/K^T projections and the output
    projection are dripped between attention k-chunks so the PE fills the
    gaps of the ACT-bound exp stream.
"""

import numpy as np

HIDDEN = 1024
SEQ = 2048
BATCH = 4
HEADS = 16
HG = 8  # heads per core
HD = 64  # head dim

_CACHE = {}
_TRACE = False  # test.py sets this for profiling runs
LAST_RESULT = None


def _build_nc():
    from collections import deque

    import concourse.mybir as mybir
    import concourse.tile as tile
    from concourse import bacc

    f32 = mybir.dt.float32
    f16 = mybir.dt.float16
    Exp = mybir.ActivationFunctionType.Exp

    nc = bacc.Bacc("TRN2", target_bir_lowering=False, debug=False)
    xT = nc.dram_tensor("xT", [HIDDEN, SEQ], f16, kind="ExternalInput").ap()
    wqk = nc.dram_tensor("wqk", [HIDDEN, 1024], f16, kind="ExternalInput").ap()
    wv = nc.dram_tensor("wv", [HIDDEN, 512], f16, kind="ExternalInput").ap()
    wout = nc.dram_tensor("wout", [512, HIDDEN], f16, kind="ExternalInput").ap()
    bqk = nc.dram_tensor("bqk", [128, 8], f32, kind="ExternalInput").ap()
    outp = nc.dram_tensor("outp", [SEQ, HIDDEN], f32, kind="ExternalOutput").ap()

    with tile.TileContext(nc) as tc:
        with (
            tc.tile_pool(name="persist", bufs=1) as pers,
            tc.tile_pool(name="pp", bufs=10) as pppool,
            tc.tile_pool(name="small", bufs=2) as small,
            tc.tile_pool(name="obuf", bufs=3) as obuf,
            tc.tile_pool(name="ps_sc", bufs=2, space="PSUM") as ps_sc,
            tc.tile_pool(name="ps_ac", bufs=1, space="PSUM") as ps_ac,
            tc.tile_pool(name="ps_aux", bufs=2, space="PSUM") as ps_aux,
        ):
            xt16 = pers.tile([128, 8, SEQ], f16, tag="xt16", name="xt16")
            wqk16 = pers.tile([128, 8, 1024], f16, tag="wqk16", name="wqk16")
            wv16 = pers.tile([128, 8, 512], f16, tag="wv16", name="wv16")
            qk = [pers.tile([128, SEQ], f16, tag=f"qk{i}", name=f"qk{i}") for i in range(8)]
            vt = [pers.tile([128, HG, 128], f16, tag=f"vt{i}", name=f"vt{i}") for i in range(16)]
            aot = [pers.tile([128, SEQ], f16, tag=f"aot{i}", name=f"aot{i}") for i in range(4)]
            wout_sb = [pers.tile([128, HIDDEN], f16, tag=f"wo{i}", name=f"wo{i}") for i in range(4)]
            bqk_sb = pers.tile([128, 8], f32, tag="bqk", name="bqk")
            ones8 = pers.tile([128, HG], f16, tag="ones8", name="ones8")

            # input DMAs; V-path first so V matmuls can start early
            for hc in range(8):
                nc.sync.dma_start(wv16[:, hc, :], wv[hc * 128 : (hc + 1) * 128, :])
            for tt in range(4):
                for hc in range(8):
                    nc.sync.dma_start(
                        xt16[:, hc, tt * 512 : (tt + 1) * 512],
                        xT[hc * 128 : (hc + 1) * 128, tt * 512 : (tt + 1) * 512],
                    )
            for hc in range(8):
                nc.sync.dma_start(wqk16[:, hc, :], wqk[hc * 128 : (hc + 1) * 128, :])
            nc.sync.dma_start(bqk_sb[:], bqk[:])
            for i in range(4):
                nc.sync.dma_start(wout_sb[i][:], wout[i * 128 : (i + 1) * 128, :])
            nc.vector.memset(ones8[:], 1.0)
            for t in range(16):
                nc.vector.memset(vt[t][:, :, HD + 1 : 128], 0.0)
                nc.vector.tensor_copy(vt[t][:, :, HD], ones8[:])

            def aux_psum():
                return ps_aux.tile([128, 512], f32, tag="aux", name="aux")

            # ---- emission-step builders (dripped between attention groups) ----
            def qk_ops(pair):
                ops = []
                for tt in range(4):
                    for ccx in (4 + pair, pair):
                        cell = {}

                        def mk_mm(hc, ccx=ccx, tt=tt, cell=cell):
                            def f():
                                if "ps" not in cell:
                                    cell["ps"] = aux_psum()
                                nc.tensor.matmul(
                                    cell["ps"][:],
                                    wqk16[:, hc, ccx * 128 : (ccx + 1) * 128],
                                    xt16[:, hc, tt * 512 : (tt + 1) * 512],
                                    start=(hc == 0),
                                    stop=(hc == 7),
                                )
                            return f

                        for hc in range(8):
                            ops.append(mk_mm(hc))

                        def mk_ev(ccx=ccx, tt=tt, cell=cell):
                            def f():
                                nc.vector.tensor_scalar_add(
                                    qk[ccx][:, tt * 512 : (tt + 1) * 512],
                                    cell["ps"][:],
                                    bqk_sb[:, ccx : ccx + 1],
                                )
                            return f

                        ops.append(mk_ev())
                return ops

            def outproj_ops(qt):
                ops = []
                for t4 in range(4):
                    tch = qt * 4 + t4
                    for nt_ in range(2):
                        cell = {}

                        def mk_mm(pair_, tch=tch, nt_=nt_, cell=cell):
                            def f():
                                if "ps" not in cell:
                                    cell["ps"] = aux_psum()
                                nc.tensor.matmul(
                                    cell["ps"][:],
                                    aot[pair_][:, tch * 128 : (tch + 1) * 128],
                                    wout_sb[pair_][:, nt_ * 512 : (nt_ + 1) * 512],
                                    start=(pair_ == 0),
                                    stop=(pair_ == 3),
                                )
                            return f

                        for pair_ in range(4):
                            ops.append(mk_mm(pair_))

                        def mk_out(tch=tch, nt_=nt_, cell=cell):
                            def f():
                                ot = obuf.tile([128, 512], f32, tag="ot", name="ot")
                                nc.vector.tensor_copy(ot[:], cell["ps"][:])
                                nc.sync.dma_start(
                                    outp[
                                        tch * 128 : (tch + 1) * 128,
                                        nt_ * 512 : (nt_ + 1) * 512,
                                    ],
                                    ot[:],
                                )
                            return f

                        ops.append(mk_out())
                return ops

            pending = deque()

            def drip(n):
                for _ in range(n):
                    if not pending:
                        return
                    pending.popleft()()

            def pace(budget, g, total_groups=64):
                return (budget * (g + 1)) // total_groups - (budget * g) // total_groups

            # ---- V projection (token-major, ones column pre-set) ----
            for t in range(16):
                pv = aux_psum()
                for hc in range(8):
                    nc.tensor.matmul(
                        pv[:],
                        xt16[:, hc, t * 128 : (t + 1) * 128],
                        wv16[:, hc, :],
                        start=(hc == 0),
                        stop=(hc == 7),
                    )
                nc.vector.tensor_copy(
                    vt[t][:, :, 0:HD], pv[:].rearrange("p (h d) -> p h d", h=HG)
                )

            # ---- Q^T/K^T for pair 0 eagerly ----
            for op in qk_ops(0):
                op()

            # ---- attention, with next-pair QK and out-proj dripped in ----
            for pair in range(4):
                if pair < 3:
                    pending.extend(qk_ops(pair + 1))
                drip_budget = len(pending)
                gidx = 0
                qtile = qk[pair]
                ktile = qk[4 + pair]
                for qt in range(4):
                    accA = ps_ac.tile([128, 512], f32, tag="accA", name="accA")
                    accB = ps_ac.tile([128, 512], f32, tag="accB", name="accB")
                    for kc in range(16):
                        sc = ps_sc.tile([128, 1024], f32, tag="sc", name="sc")
                        nc.tensor.matmul(
                            sc[:, 0:512],
                            ktile[0:64, kc * 128 : (kc + 1) * 128],
                            qtile[0:64, qt * 512 : (qt + 1) * 512],
                            start=True,
                            stop=True,
                            tile_position=(0, 0),
                        )
                        nc.tensor.matmul(
                            sc[:, 512:1024],
                            ktile[64:128, kc * 128 : (kc + 1) * 128],
                            qtile[64:128, qt * 512 : (qt + 1) * 512],
                            start=True,
                            stop=True,
                            tile_position=(64, 0),
                        )
                        pp = pppool.tile([128, 1024], f16, tag="pp", name="pp")
                        nc.scalar.activation(pp[:], sc[:], Exp)
                        nc.tensor.matmul(
                            accA[:],
                            vt[kc][:, 2 * pair, :],
                            pp[:, 0:512],
                            start=(kc == 0),
                            stop=(kc == 15),
                        )
                        nc.tensor.matmul(
                            accB[:],
                            vt[kc][:, 2 * pair + 1, :],
                            pp[:, 512:1024],
                            start=(kc == 0),
                            stop=(kc == 15),
                        )
                        drip(pace(drip_budget, gidx) + (3 if pair == 3 else 0))
                        gidx += 1
                    # normalization: read PSUM out early so acc banks free fast
                    parts = []
                    for acc, row0 in ((accA, 0), (accB, 64)):
                        dn = small.tile([1, 512], f32, tag="dn", name="dn")
                        nc.vector.tensor_copy(dn[:], acc[64:65, :])
                        numer = small.tile([64, 512], f32, tag="numer", name="numer")
                        nc.vector.tensor_copy(numer[:], acc[0:64, :])
                        parts.append((dn, numer, row0))
                    for dn, numer, row0 in parts:
                        bc = small.tile([64, 512], f32, tag="bc", name="bc")
                        nc.gpsimd.partition_broadcast(bc[:], dn[:])
                        rc = small.tile([64, 512], f32, tag="rc", name="rc")
                        scr = small.tile([64, 512], f32, tag="scr", name="scr")
                        nc.vector.reciprocal_approx_accurate(rc[:], bc[:], scr[:])
                        nc.vector.tensor_mul(
                            aot[pair][row0 : row0 + 64, qt * 512 : (qt + 1) * 512],
                            numer[:],
                            rc[:],
                        )
                    if pair == 3:
                        pending.extend(outproj_ops(qt))
            while pending:
                pending.popleft()()
    nc.compile()
    return nc


def _get_nc():
    if "nc" not in _CACHE:
        _CACHE["nc"] = _build_nc()
    return _CACHE["nc"]


def kernel(x, W_qkv, b_qkv, W_out, b_out):
    global LAST_RESULT
    from concourse.bass_utils import run_bass_kernel_spmd

    x = np.asarray(x, dtype=np.float32)
    W_qkv = np.asarray(W_qkv, dtype=np.float32)
    b_qkv = np.asarray(b_qkv, dtype=np.float32)
    W_out = np.asarray(W_out, dtype=np.float32)
    b_out = np.asarray(b_out, dtype=np.float32)

    scale = 1.0 / np.sqrt(HD)
    # [hidden, 3, heads, hd]
    w4 = W_qkv.reshape(HIDDEN, 3, HEADS, HD)
    b4 = b_qkv.reshape(3, HEADS, HD)

    in_maps = []
    for c in range(8):
        b = c // 2
        g = c % 2
        hs = slice(g * HG, (g + 1) * HG)
        wq = (w4[:, 0, hs, :] * scale).reshape(HIDDEN, 512)
        wk = w4[:, 1, hs, :].reshape(HIDDEN, 512)
        wv_ = np.ascontiguousarray(w4[:, 2, hs, :].reshape(HIDDEN, 512)).astype(
            np.float16
        )
        wqk = np.ascontiguousarray(np.concatenate([wq, wk], axis=1)).astype(np.float16)
        bq = (b4[0, hs, :] * scale).reshape(512)
        bk = b4[1, hs, :].reshape(512)
        bqk = np.ascontiguousarray(
            np.concatenate([bq, bk]).reshape(8, 128).T
        ).astype(np.float32)
        wout_c = np.ascontiguousarray(W_out[g * 512 : (g + 1) * 512, :]).astype(
            np.float16
        )
        xT_b = np.ascontiguousarray(x[b].T).astype(np.float16)
        in_maps.append(
            {
                "xT": xT_b,
                "wqk": wqk,
                "wv": wv_,
                "wout": wout_c,
                "bqk": bqk,
            }
        )

    nc = _get_nc()
    res = run_bass_kernel_spmd(
        nc, in_maps, core_ids=list(range(8)), trace=_TRACE
    )
    LAST_RESULT = res

    # host reduction: sum the two head-group partials per batch; fold V-bias
    # and output bias (adding b_v to V shifts every attn output row by b_v,
    # which after the out-projection is the constant b_v @ W_out).
    bv_all = b_qkv[2 * HIDDEN : 3 * HIDDEN]
    const = (b_out + bv_all @ W_out).astype(np.float32)
    out = np.empty((BATCH, SEQ, HIDDEN), dtype=np.float32)
    for b in range(BATCH):
        out[b] = res.results[2 * b]["outp"] + res.results[2 * b + 1]["outp"] + const
    return out
